# revision 6
# baseline (speedup 1.0000x reference)
"""Builder for the per-core attention kernel.

Math (per core, one batch element, T tokens, H=1024, nH=8, hd=128):
  Q = x @ wq.T + bq ; K = x @ wk.T + bk ; V = x @ wv.T + bv
  scores[t,q,k] = sum_d Q[t,q*128+d] K[t,k*128+d] / sqrt(128)   (per-position)
  attn = softmax_k(scores)
  ctx[t, q*128+d] = sum_k attn[t,q,k] V[t, k*128+d]
  out = ctx @ wf.T + bf

Layout strategy (bf16 matmuls, f32 accum):
  - xT tiles [h, t] loaded via HW DMA-transpose from DRAM
  - Q^T, K^T per-head tiles [d, t] via weight-stationary matmuls
  - V natural rows [t, o] via xT-stationary matmuls
  - scores: pack 16 positions x 8 heads onto the 128-partition contraction:
    one 128x128 matmul per 16-position group gives all (q,k) pairs for all
    16 positions; masked mul + strided reduce extracts the diagonal blocks
  - softmax over k=8, batched over a whole chunk on DVE/ACT
  - context: block-diag attn (built via PE transpose + replicate-DMA + mask)
    against grouped V (built via SBUF->SBUF rearrange DMA); one matmul per
    group yields ctx^T grouped [d, (q, s')]
  - FC: ctx^T-stationary matmuls -> natural out rows [t, o] + bias
"""

import sys

if "/opt/trn_rl_repo" not in sys.path:
    sys.path.insert(0, "/opt/trn_rl_repo")

from contextlib import ExitStack

import numpy as np
import ml_dtypes

import concourse.bass as bass
import concourse.tile as tile
from concourse import bacc, mybir
from concourse.bass_utils import run_bass_kernel_spmd

F32 = mybir.dt.float32
BF16 = mybir.dt.bfloat16

H = 1024
NH = 8
HD = 128
GP = 16           # positions per group
GPB = 4           # groups per psum batch (4*128 = 512 free)


def host_inputs(x_b, wq, bq, wk, bk, wv, bv, wf, bf):
    """Per-core host-side input prep. x_b: [T, H] float32 (one batch row)."""
    bft = ml_dtypes.bfloat16
    d = {}
    d["x"] = np.ascontiguousarray(x_b.astype(bft))
    d["wqT"] = np.ascontiguousarray(wq.T.astype(bft))
    d["wkT"] = np.ascontiguousarray(wk.T.astype(bft))
    d["wvT"] = np.ascontiguousarray(wv.T.astype(bft))
    d["wfT"] = np.ascontiguousarray(wf.T.astype(bft))
    # Q/K bias per (d, head): partition = d, column = head
    d["bqT"] = np.ascontiguousarray(bq.reshape(NH, HD).T.astype(np.float32))
    d["bkT"] = np.ascontiguousarray(bk.reshape(NH, HD).T.astype(np.float32))
    # V/FC bias broadcast across partitions
    d["bvb"] = np.ascontiguousarray(np.tile(bv.astype(np.float32), (128, 1)))
    d["bfb"] = np.ascontiguousarray(np.tile(bf.astype(np.float32), (128, 1)))
    # scores extraction mask [128, GPB*NH*GP]: p = q*16+s', f = (G, k, s'')
    ms = np.zeros((128, GPB, NH, GP), dtype=np.float32)
    for p in range(128):
        ms[p, :, :, p % GP] = 1.0 / np.sqrt(HD)
    d["maskS"] = ms.reshape(128, GPB * NH * GP)
    # A mask [128, GPB*128]: p = k*16+s'', f = (G, q, s')
    ma = np.zeros((128, GPB, NH, GP), dtype=bft)
    for p in range(128):
        ma[p, :, :, p % GP] = 1.0
    d["maskA"] = ma.reshape(128, GPB * 128)
    d["ident"] = np.eye(128, dtype=bft)
    return d


def declare_io(nc, T):
    io = {}
    io["x"] = nc.dram_tensor("x", [T, H], BF16, kind="ExternalInput").ap()
    for n in ("wqT", "wkT", "wvT", "wfT"):
        io[n] = nc.dram_tensor(n, [H, H], BF16, kind="ExternalInput").ap()
    for n in ("bqT", "bkT"):
        io[n] = nc.dram_tensor(n, [HD, NH], F32, kind="ExternalInput").ap()
    for n in ("bvb", "bfb"):
        io[n] = nc.dram_tensor(n, [128, H], F32, kind="ExternalInput").ap()
    io["maskS"] = nc.dram_tensor("maskS", [128, GPB * NH * GP], F32, kind="ExternalInput").ap()
    io["maskA"] = nc.dram_tensor("maskA", [128, GPB * 128], BF16, kind="ExternalInput").ap()
    io["ident"] = nc.dram_tensor("ident", [128, 128], BF16, kind="ExternalInput").ap()
    io["out"] = nc.dram_tensor("out", [T, H], F32, kind="ExternalOutput").ap()
    io["attn"] = nc.dram_tensor("attn", [T, NH, NH], F32, kind="ExternalOutput").ap()
    return io


def build(ctx: ExitStack, tc: tile.TileContext, io, T, TC=512, reps=1):
    """Emit the kernel body. reps>1 re-emits the whole computation (for timing)."""
    nc = tc.nc
    NCH = T // TC          # chunks
    NSUB = TC // 128       # 128-token subtiles per chunk
    NG = TC // GP          # groups per chunk
    NB = NG // GPB         # psum batches per chunk

    wpool = ctx.enter_context(tc.tile_pool(name="weights", bufs=1))
    cpool = ctx.enter_context(tc.tile_pool(name="chunk", bufs=1))
    xpool = ctx.enter_context(tc.tile_pool(name="xin", bufs=2))
    opool = ctx.enter_context(tc.tile_pool(name="outb", bufs=4))
    spool = ctx.enter_context(tc.tile_pool(name="small", bufs=2))
    dpool = ctx.enter_context(tc.tile_pool(name="dram", bufs=2, space="DRAM"))
    psum = ctx.enter_context(tc.tile_pool(name="psum", bufs=4, space="PSUM"))
    psumT = ctx.enter_context(tc.tile_pool(name="psumT", bufs=2, space="PSUM"))

    # persistent weights/consts
    wT = {}
    for n in ("wqT", "wkT", "wvT", "wfT"):
        w = wpool.tile([128, 8 * H], BF16, name=n + "_sb")
        # DRAM [H, H] -> [128, (htile, o)]
        nc.sync.dma_start(w[:].rearrange("p (i o) -> p i o", o=H),
                          io[n].rearrange("(i p) o -> p i o", p=128))
        wT[n] = w
    bqT = wpool.tile([128, NH], F32)
    nc.sync.dma_start(bqT[:], io["bqT"][:])
    bkT = wpool.tile([128, NH], F32)
    nc.sync.dma_start(bkT[:], io["bkT"][:])
    bvb = wpool.tile([128, H], F32)
    nc.sync.dma_start(bvb[:], io["bvb"][:])
    bfb = wpool.tile([128, H], F32)
    nc.sync.dma_start(bfb[:], io["bfb"][:])
    maskS = wpool.tile([128, GPB * NH * GP], F32)
    nc.sync.dma_start(maskS[:], io["maskS"][:])
    maskA = wpool.tile([128, GPB * 128], BF16)
    nc.sync.dma_start(maskA[:], io["maskA"][:])
    ident = wpool.tile([128, 128], BF16)
    nc.sync.dma_start(ident[:], io["ident"][:])

    for rep in range(reps):
        for c in range(NCH):
            c0 = c * TC

            # ---- xT via DMA transpose: [128 (h in tile), (htile, t)] ----
            xT = xpool.tile([128, 8 * TC], BF16)
            for h in range(8):
                nc.sync.dma_start_transpose(
                    out=xT[:, h * TC:(h + 1) * TC],
                    in_=io["x"][c0:c0 + TC, h * 128:(h + 1) * 128])

            # ---- Q^T, K^T (weight-stationary), stored GROUPED:
            # column = g*128 + j*16 + s'  (per-group [128,128] stationary tiles)
            QT = cpool.tile([128, 8 * TC], BF16)
            KT = cpool.tile([128, 8 * TC], BF16)
            for (wname, bT, dst) in (("wqT", bqT, QT), ("wkT", bkT, KT)):
                dstv = dst[:].rearrange("p (g j s) -> p g j s", j=NH, s=GP)
                for j in range(8):   # head / o-tile
                    for n in range(TC // 512):
                        pq = psum.tile([128, 512], F32, tag="mm")
                        for h in range(8):
                            nc.tensor.matmul(
                                pq[:],
                                lhsT=wT[wname][:, h * H + j * 128: h * H + j * 128 + 128],
                                rhs=xT[:, h * TC + n * 512: h * TC + n * 512 + 512],
                                start=(h == 0), stop=(h == 7))
                        nc.scalar.activation(
                            dstv[:, n * 32:(n + 1) * 32, j, :],
                            pq[:].rearrange("p (g s) -> p g s", s=GP),
                            mybir.ActivationFunctionType.Identity,
                            bias=bT[:, j:j + 1], scale=1.0)

            # ---- V natural rows (xT-stationary): [128 t, (sub, o)] ----
            Vnat = cpool.tile([128, NSUB * H], BF16)
            for sub in range(NSUB):
                pv = [psum.tile([128, 512], F32, tag="mm", name=f"pv{n}") for n in range(2)]
                for h in range(8):
                    for n in range(2):
                        nc.tensor.matmul(
                            pv[n][:],
                            lhsT=xT[:, h * TC + sub * 128: h * TC + sub * 128 + 128],
                            rhs=wT["wvT"][:, h * H + n * 512: h * H + n * 512 + 512],
                            start=(h == 0), stop=(h == 7))
                for n in range(2):
                    nc.vector.tensor_tensor(
                        out=Vnat[:, sub * H + n * 512: sub * H + n * 512 + 512],
                        in0=pv[n][:], in1=bvb[:, n * 512:n * 512 + 512],
                        op=mybir.AluOpType.add)

            # ---- scores + extraction ----
            scores = cpool.tile([128, NG * NH], F32)     # [p=(q,s'), (g,k)]
            for b in range(NB):
                ps = psum.tile([128, GPB * 128], F32, tag="mm")
                for i in range(GPB):
                    g = b * GPB + i
                    nc.tensor.matmul(
                        ps[:, i * 128:(i + 1) * 128],
                        lhsT=QT[:, g * 128:(g + 1) * 128],
                        rhs=KT[:, g * 128:(g + 1) * 128],
                        start=True, stop=True)
                scr = spool.tile([128, GPB * 128], F32, tag="scr")
                nc.vector.tensor_tensor(out=scr[:], in0=ps[:], in1=maskS[:],
                                        op=mybir.AluOpType.mult)
                nc.vector.tensor_reduce(
                    out=scores[:, b * GPB * NH:(b + 1) * GPB * NH],
                    in_=scr[:].rearrange("p (G k s) -> p G k s", k=NH, s=GP),
                    axis=mybir.AxisListType.X, op=mybir.AluOpType.add)

            # ---- softmax over k (free innermost) ----
            scv = scores[:].rearrange("p (g k) -> p g k", k=NH)
            mx = spool.tile([128, NG], F32)
            nc.vector.tensor_reduce(out=mx[:], in_=scv, axis=mybir.AxisListType.X,
                                    op=mybir.AluOpType.max)
            shifted = spool.tile([128, NG * NH], F32)
            nc.vector.tensor_tensor(
                out=shifted[:].rearrange("p (g k) -> p g k", k=NH), in0=scv,
                in1=mx[:, :, None].broadcast_to([128, NG, NH]),
                op=mybir.AluOpType.subtract)
            ex = spool.tile([128, NG * NH], F32)
            nc.scalar.activation(ex[:], shifted[:], mybir.ActivationFunctionType.Exp)
            sm = spool.tile([128, NG], F32)
            nc.vector.tensor_reduce(out=sm[:], in_=ex[:].rearrange("p (g k) -> p g k", k=NH),
                                    axis=mybir.AxisListType.X, op=mybir.AluOpType.add)
            rc = spool.tile([128, NG], F32)
            nc.vector.reciprocal(rc[:], sm[:])
            attn = spool.tile([128, NG * NH], F32)
            nc.vector.tensor_tensor(
                out=attn[:].rearrange("p (g k) -> p g k", k=NH),
                in0=ex[:].rearrange("p (g k) -> p g k", k=NH),
                in1=rc[:, :, None].broadcast_to([128, NG, NH]),
                op=mybir.AluOpType.mult)

            # ---- attn -> DRAM [t, q, k] ----
            for q in range(NH):
                nc.sync.dma_start(
                    out=io["attn"][c0:c0 + TC, q, :].rearrange("(g s) k -> s g k", s=GP),
                    in_=attn[q * GP:(q + 1) * GP, :].rearrange("s (g k) -> s g k", k=NH))

            # ---- block-diag attn stationary: transpose + replicate + mask ----
            attnb = spool.tile([128, NG * NH], BF16)
            nc.vector.tensor_copy(attnb[:], attn[:])
            paT = psumT.tile([128, (NG // 16) * 128], BF16, tag="tpose")
            for half in range(NG // 16):
                nc.tensor.transpose(
                    out=paT[:, half * 128:(half + 1) * 128],
                    in_=attnb[:, half * 128:(half + 1) * 128],
                    identity=ident[:])
            attnT = spool.tile([128, (NG // 16) * 128], BF16)   # [p=(g16,k), (q,s')]
            nc.vector.tensor_copy(attnT[:], paT[:])

            # replicate via DRAM roundtrip: attnT rows (g,k) -> Ag[(k,s''), (g,f)]
            ascr = dpool.tile([NG * NH, 128], BF16)
            for half in range(NG // 16):
                nc.sync.dma_start(ascr[half * 128:(half + 1) * 128, :],
                                  attnT[:, half * 128:(half + 1) * 128])
            ascrv = ascr[:].rearrange("(g k) f -> k g f", k=NH)
            Ag = cpool.tile([128, NG * 128], BF16)   # [p=(k,s''), (g, q, s')]
            for k in range(NH):
                nc.sync.dma_start(
                    out=Ag[k * GP:(k + 1) * GP, :].rearrange("s (g f) -> s g f", f=128),
                    in_=ascrv[k][None, :, :].broadcast_to([GP, NG, 128]))
            for b in range(NB):
                nc.vector.tensor_tensor(
                    out=Ag[:, b * GPB * 128:(b + 1) * GPB * 128],
                    in0=Ag[:, b * GPB * 128:(b + 1) * GPB * 128],
                    in1=maskA[:], op=mybir.AluOpType.mult)

            # ---- grouped V via DRAM roundtrip: Vg[(k,s''), (g, d)] ----
            vscr = dpool.tile([NH, TC, 128], BF16)
            for k in range(NH):
                nc.sync.dma_start(
                    out=vscr[k].rearrange("(u p) d -> p u d", u=NSUB),
                    in_=Vnat[:].rearrange("p (u o) -> p u o", o=H)[:, :, k * 128:(k + 1) * 128])
            Vg = cpool.tile([128, NG * 128], BF16)   # [p=(k,s''), (g, d)]
            for k in range(NH):
                nc.sync.dma_start(
                    out=Vg[k * GP:(k + 1) * GP, :].rearrange("s (g d) -> s g d", d=128),
                    in_=vscr[k].rearrange("(g s) d -> s g d", s=GP))

            # ---- context matmuls: ctx^T stored FC-ready:
            # column = sub*1024 + q*128 + gin*16 + s'  (gin = group within sub)
            ctxT = cpool.tile([128, NSUB * H], BF16)
            ctxv = ctxT[:].rearrange("p (u q g s) -> p u q g s", q=NH, g=8, s=GP)
            for b in range(NB):
                pc = psum.tile([128, GPB * 128], F32, tag="mm")
                for i in range(GPB):
                    g = b * GPB + i
                    nc.tensor.matmul(
                        pc[:, i * 128:(i + 1) * 128],
                        lhsT=Vg[:, g * 128:(g + 1) * 128],
                        rhs=Ag[:, g * 128:(g + 1) * 128],
                        start=True, stop=True)
                gb = (b * GPB) % 8
                nc.scalar.activation(
                    ctxv[:, (b * GPB) // 8, :, gb:gb + GPB, :],
                    pc[:].rearrange("p (g q s) -> p q g s", q=NH, s=GP),
                    mybir.ActivationFunctionType.Copy)

            # ---- FC (ctx^T-stationary) -> natural out rows + bias ----
            for sub in range(NSUB):
                pf = [psum.tile([128, 512], F32, tag="mm", name=f"pf{n}") for n in range(2)]
                for q in range(NH):
                    for n in range(2):
                        nc.tensor.matmul(
                            pf[n][:],
                            lhsT=ctxT[:, sub * H + q * 128: sub * H + q * 128 + 128],
                            rhs=wT["wfT"][:, q * H + n * 512: q * H + n * 512 + 512],
                            start=(q == 0), stop=(q == 7))
                osb = opool.tile([128, H], F32, tag="osb")
                for n in range(2):
                    nc.vector.tensor_tensor(
                        out=osb[:, n * 512:(n + 1) * 512], in0=pf[n][:],
                        in1=bfb[:, n * 512:n * 512 + 512], op=mybir.AluOpType.add)
                nc.sync.dma_start(out=io["out"][c0 + sub * 128: c0 + (sub + 1) * 128, :],
                                  in_=osb[:])


def build_nc(T, TC=512, reps=1):
    nc = bacc.Bacc("TRN2", target_bir_lowering=False, debug=False)
    io = declare_io(nc, T)
    with tile.TileContext(nc) as tc:
        with ExitStack() as ctx:
            build(ctx, tc, io, T, TC=TC, reps=reps)
    nc.compile()
    return nc


S = 4096
B = 8
_NC_CACHE = {}


def _get_nc(reps=1):
    if reps not in _NC_CACHE:
        _NC_CACHE[reps] = build_nc(S, TC=512, reps=reps)
    return _NC_CACHE[reps]


def make_in_maps(x, wq, bq, wk, bk, wv, bv, wf, bf):
    """Per-core input maps: core i gets batch row i (data-parallel)."""
    x = np.asarray(x)
    shared = host_inputs(np.zeros((1, H), np.float32), np.asarray(wq), np.asarray(bq),
                         np.asarray(wk), np.asarray(bk), np.asarray(wv), np.asarray(bv),
                         np.asarray(wf), np.asarray(bf))
    del shared["x"]
    bft = ml_dtypes.bfloat16
    in_maps = []
    for i in range(B):
        m = dict(shared)
        m["x"] = np.ascontiguousarray(x[i].astype(bft))
        in_maps.append(m)
    return in_maps


def kernel(x, wq, bq, wk, bk, wv, bv, wf, bf, _reps=1):
    nc = _get_nc(_reps)
    in_maps = make_in_maps(x, wq, bq, wk, bk, wv, bv, wf, bf)
    res = run_bass_kernel_spmd(nc, in_maps, list(range(B)))
    out = np.stack([res.results[i]["out"] for i in range(B)])
    attn = np.stack([res.results[i]["attn"] for i in range(B)])
    return out, attn
